# revision 7
# baseline (speedup 1.0000x reference)
"""Trainium2 Bass kernel for a GRU encoder-decoder (KLCPD generator).

Model (see reference):
  past_emb = relu(past @ W_emb + b_emb)            [T,B,E]
  fut_emb  = relu(future @ W_emb + b_emb)          [T,B,E]
  _, h_T   = GRU_enc(past_emb, h0=0)
  hidden   = h_T + noise
  ys, _    = GRU_dec(shift(fut_emb), h0=hidden)
  out      = ys @ W_out + b_out                    [T,B,D]

Sharding: data-parallel over batch B=1024 across 8 NeuronCores
(B_local=128); all weights replicated; no collectives.

Per-core kernel layout decisions:
  * All matmul inputs are bf16 (fp32 accumulation in PSUM); inputs and
    weights are cast to bf16 on the HOST, halving DMA traffic and
    removing all on-device staging casts.
  * The GRU hidden state is kept *transposed* in SBUF as
    hT[p, k*128 + b] = h[b, k*128 + p]  (k = H-chunk 0..3), so the
    elementwise gate math produces, with zero extra transposes, exactly
    the stationary operand needed by the next step's matmuls.
  * Per step the PE emits bank-major waves (all 16 r matmuls, then z,
    then hn, then step t+1's input projections), so sigmoid(r) — the
    head of the serial gate chain — unblocks ~0.9us into the wave.
  * Gate tail: h = n + z*(h_prev - n), all plain 2x-mode DVE
    tensor-tensor ops on bf16; sigmoids full-width, the rest H-halved.
  * The decoder's output projection out_t = h_t @ W_out rides the GRU
    loop as 4 extra matmuls per step (the transposed state IS the
    needed lhsT), accumulating 4 steps per PSUM bank; no DRAM
    round-trip of the decoder states.
"""

import os
from contextlib import ExitStack

import numpy as np

import concourse.bass as bass
import concourse.tile as tile
from concourse import bacc, bass_utils, masks, mybir
from concourse.tile_rust import add_dep_helper

T, B, D, E, H = 64, 1024, 128, 256, 512
NCORES = 8
BL = B // NCORES  # 128
H3 = 3 * H
P = 128

f32 = mybir.dt.float32
bf16 = mybir.dt.bfloat16
NP_BF16 = mybir.dt.np(bf16)
AF = mybir.ActivationFunctionType
OP = mybir.AluOpType


def _mm(nc, out, lhsT, rhs, start, stop):
    return nc.tensor.matmul(out, lhsT, rhs, start=start, stop=stop,
                            skip_group_check=True)


# The single-buffered phn bank is protected by tile's slice-level
# read/write ordering (PSUM start=True clears has_written bits, not data);
# the explicit cross-step dep deadlocks the tile scheduler.
USE_PHN_DEP = False


def build_module(zero_bias: bool, t_steps: int = T):
    """Builds the per-core Bass module. Returns the compiled nc."""
    nc = bacc.Bacc("TRN2", target_bir_lowering=False, debug=False)

    past = nc.dram_tensor("past", [t_steps, BL, D], bf16, kind="ExternalInput").ap()
    fut = nc.dram_tensor("fut", [t_steps, BL, D], bf16, kind="ExternalInput").ap()
    noise = nc.dram_tensor("noise", [BL, H], bf16, kind="ExternalInput").ap()
    w_emb = nc.dram_tensor("w_emb", [D, E], bf16, kind="ExternalInput").ap()
    b_emb = nc.dram_tensor("b_emb", [1, E], f32, kind="ExternalInput").ap()
    wd = {}
    for g in ("enc", "dec"):
        wd[g, "ih"] = nc.dram_tensor(f"w_ih_{g}", [E, H3], bf16, kind="ExternalInput").ap()
        wd[g, "hh"] = nc.dram_tensor(f"w_hh_{g}", [H, H3], bf16, kind="ExternalInput").ap()
        wd[g, "bih"] = nc.dram_tensor(f"b_ih_{g}", [1, H3], f32, kind="ExternalInput").ap()
        wd[g, "bhh"] = nc.dram_tensor(f"b_hh_{g}", [1, H3], f32, kind="ExternalInput").ap()
    w_out = nc.dram_tensor("w_out", [H, D], bf16, kind="ExternalInput").ap()
    b_out = nc.dram_tensor("b_out", [1, D], f32, kind="ExternalInput").ap()
    out = nc.dram_tensor("out", [t_steps, BL, D], f32, kind="ExternalOutput").ap()

    with tile.TileContext(nc, pool_alloc_mode="queue") as tc, ExitStack() as octx:
        wpool = octx.enter_context(tc.tile_pool(name="weights", bufs=1))

        # ---- constants -------------------------------------------------
        ident = wpool.tile([P, P], bf16)
        masks.make_identity(nc, ident[:])
        ones_row = wpool.tile([1, 512], bf16)
        nc.gpsimd.memset(ones_row[:], 1.0)

        # ---- small weights needed first (program order = DMA order) ----
        wemb_bf = wpool.tile([P, E], bf16)
        nc.sync.dma_start(wemb_bf[:], w_emb[:, :])
        nbf = wpool.tile([P, H], bf16, tag="nstage")
        nc.sync.dma_start(nbf[:], noise[:, :])

        bemb_bf = None
        if not zero_bias:
            with tc.tile_pool(name="bstage", bufs=1) as bstage:
                st = bstage.tile([1, E], f32, tag="s_bemb")
                nc.sync.dma_start(st[:], b_emb[:, :])
                bemb_bf = wpool.tile([1, E], bf16)
                nc.vector.tensor_copy(bemb_bf[:], st[:])

        # noise, transposed: noiseT[p, k*128+b] = noise[b, k*128+p]
        noiseT = wpool.tile([P, H], bf16)
        with tc.tile_pool(name="psum_noise", bufs=1, space="PSUM") as pn:
            pt = pn.tile([P, H], bf16)
            for k in range(4):
                nc.tensor.transpose(pt[:, k * P:(k + 1) * P], nbf[:, k * P:(k + 1) * P],
                                    ident[:])
            nc.scalar.copy(noiseT[:], pt[:])

        # ---- embedding precompute --------------------------------------
        # embT[g][e][p, t*BL + b] = relu(x[t] @ W_emb + b_emb)[b, e*128+p]
        # Interleaved with the bulk weight DMAs so the x transfers (needed
        # first) lead the DMA queue.
        embT = {g: [wpool.tile([P, t_steps * BL], bf16, name=f"embT_{g}_{e}",
                               tag=f"embT_{g}_{e}")
                    for e in range(2)]
                for g in ("enc", "dec")}
        whh = {}   # whh[g][k]: [128, H3]
        wih = {}   # wih[g][e]: [128, H3]
        biasx = {}  # [1, H3]  (b_ih + b_hh on r,z cols; b_ih on n cols)
        biashn = {}  # [1, 512] (b_hh n-part)
        n_grp = t_steps // 4

        def emb_phase(g, x_ap, ep, pep):
            for gi in range(n_grp):
                xs = ep.tile([P, 4 * P], bf16, tag="xs")
                nc.sync.dma_start(
                    xs[:].rearrange("p (i d) -> p i d", i=4),
                    x_ap[4 * gi:4 * gi + 4].transpose([1, 0, 2]),
                )
                ptr = pep.tile([P, 4 * P], bf16, tag="ptr")
                for i in range(4):
                    nc.tensor.transpose(ptr[:, i * P:(i + 1) * P],
                                        xs[:, i * P:(i + 1) * P], ident[:])
                xT = ep.tile([P, 4 * P], bf16, tag="xT")
                nc.scalar.copy(xT[:], ptr[:])
                for e in range(2):
                    pe_ = pep.tile([P, 4 * P], f32, tag=f"pe{e}")
                    _mm(nc, pe_[:], wemb_bf[:, e * P:(e + 1) * P], xT[:],
                        start=True, stop=zero_bias)
                    if not zero_bias:
                        _mm(nc, pe_[:], bemb_bf[0:1, e * P:(e + 1) * P],
                            ones_row[0:1, :], start=False, stop=True)
                    dst = embT[g][e][:, gi * 4 * P:(gi + 1) * 4 * P]
                    if e == 0:
                        nc.scalar.activation(dst, pe_[:], AF.Relu)
                    else:
                        nc.vector.tensor_scalar_max(dst, pe_[:], 0.0)

        def load_gru_weights(g):
            wih[g] = []
            for e in range(2):
                t_ = wpool.tile([P, H3], bf16, tag=f"wih_{g}_{e}")
                nc.sync.dma_start(t_[:], wd[g, "ih"][e * P:(e + 1) * P, :])
                wih[g].append(t_)
            whh[g] = []
            for k in range(4):
                t_ = wpool.tile([P, H3], bf16, tag=f"whh_{g}_{k}")
                nc.sync.dma_start(t_[:], wd[g, "hh"][k * P:(k + 1) * P, :])
                whh[g].append(t_)
            if not zero_bias:
                with tc.tile_pool(name=f"bstage_{g}", bufs=1) as stage:
                    sih = stage.tile([1, H3], f32, tag="s_bih")
                    shh = stage.tile([1, H3], f32, tag="s_bhh")
                    nc.sync.dma_start(sih[:], wd[g, "bih"][:, :])
                    nc.sync.dma_start(shh[:], wd[g, "bhh"][:, :])
                    bx = wpool.tile([1, H3], bf16, tag=f"biasx_{g}")
                    nc.vector.tensor_add(bx[:, 0:2 * H], sih[:, 0:2 * H], shh[:, 0:2 * H])
                    nc.vector.tensor_copy(bx[:, 2 * H:H3], sih[:, 2 * H:H3])
                    bh = wpool.tile([1, H], bf16, tag=f"biashn_{g}")
                    nc.vector.tensor_copy(bh[:], shh[:, 2 * H:H3])
                    biasx[g] = bx
                    biashn[g] = bh

        with tc.tile_pool(name="estage", bufs=4) as ep, \
             tc.tile_pool(name="psum_emb", bufs=2, space="PSUM") as pep:
            emb_phase("enc", past, ep, pep)
            load_gru_weights("enc")
            emb_phase("dec", fut, ep, pep)
            load_gru_weights("dec")
            wout_bf = wpool.tile([P, 4 * P], bf16)  # col block k = W_out rows k
            for k in range(4):
                nc.sync.dma_start(wout_bf[:, k * P:(k + 1) * P],
                                  w_out[k * P:(k + 1) * P, :])
            bout_bf = None
            if not zero_bias:
                with tc.tile_pool(name="bstage_o", bufs=1) as stage:
                    st = stage.tile([1, D], f32, tag="s_bout")
                    nc.sync.dma_start(st[:], b_out[:, :])
                    bout_bf = wpool.tile([1, D], bf16)
                    nc.vector.tensor_copy(bout_bf[:], st[:])

        # ---- GRU loops --------------------------------------------------
        last_phn_reader = [None]  # last DVE op reading the (single) phn bank

        def gru_loop(g, is_dec, hT0, sb, pg, pg1, pout, ostage):
            """Runs t_steps of GRU g. hT0 = initial transposed state (or None).
            Returns final hT tile."""
            hT_prev = hT0
            po = [None]

            def emit_xw(t):
                """Allocate step t's PSUM banks and emit its input-projection
                matmuls. Returns the bank state for the recurrent matmuls and
                gate tail."""
                have_x = (not is_dec) or t > 0
                have_h = t > 0 or hT0 is not None
                have_xn = have_x or not zero_bias
                pr = pg.tile([P, H], f32, name="pr", tag="pr")
                pz = pg.tile([P, H], f32, name="pz", tag="pz")
                pxn = pg.tile([P, H], f32, name="pxn", tag="pxn") if have_xn else None
                phn = pg1.tile([P, H], f32, name="phn", tag="phn") if have_h else None

                nbias = 0 if zero_bias else 1
                nxw = (2 if have_x else 0) + nbias
                nhw = 4 if have_h else 0
                totals = {id(pr): 4 * (nxw + nhw), id(pz): 4 * (nxw + nhw)}
                if pxn is not None:
                    totals[id(pxn)] = 4 * nxw
                if phn is not None:
                    totals[id(phn)] = 4 * (4 + nbias)
                emitted = {k: 0 for k in totals}

                def emit(bank, sl, lhsT, rhs):
                    emitted[id(bank)] += 1
                    op = _mm(nc, sl, lhsT, rhs,
                             start=emitted[id(bank)] == 1,
                             stop=emitted[id(bank)] == totals[id(bank)])
                    if (USE_PHN_DEP and bank is phn and emitted[id(bank)] == 1
                            and last_phn_reader[0] is not None):
                        # The start=True pending-zero clears the WHOLE bank;
                        # tile only tracks the written slice, so order the
                        # clear after the previous step's last phn read.
                        add_dep_helper(last_phn_reader[0].ins, op.ins, sync=True,
                                       reason="phn bank pending-zero WAR")
                    return op

                tcol = (t - 1) if is_dec else t
                lx = ([embT[g][e][:, tcol * BL:(tcol + 1) * BL] for e in range(2)]
                      if have_x else None)
                for bank, lo in ((pr, 0), (pz, H), (pxn, 2 * H)):
                    if bank is None:
                        continue
                    for m in range(4):
                        sl = bank[:, m * P:(m + 1) * P]
                        if not zero_bias:
                            emit(bank, sl, biasx[g][0:1, lo + m * P:lo + (m + 1) * P],
                                 ones_row[0:1, 0:P])
                        if have_x:
                            for e in range(2):
                                emit(bank, sl, wih[g][e][:, lo + m * P:lo + (m + 1) * P], lx[e])
                return pr, pz, pxn, phn, emit

            def emit_out(t, h_t):
                """Fused output projection: out[t] = h_t @ W_out (+ b_out),
                accumulated into quarter t%4 of the po bank."""
                i = t % 4
                if i == 0:
                    po[0] = pout.tile([P, 4 * P], f32, name="po", tag="po")
                sl = po[0][:, i * P:(i + 1) * P]
                if not zero_bias:
                    _mm(nc, sl, bout_bf[0:1, :], ones_row[0:1, 0:P],
                        start=True, stop=False)
                for k in range(4):
                    _mm(nc, sl, h_t[:, k * P:(k + 1) * P], wout_bf[:, k * P:(k + 1) * P],
                        start=zero_bias and k == 0, stop=k == 3)
                if i == 3:
                    outf = ostage.tile([P, 4 * P], f32, tag="outf")
                    nc.scalar.copy(outf[:], po[0][:])
                    w = t // 4
                    nc.sync.dma_start(
                        out[4 * w:4 * w + 4].transpose([1, 0, 2]),
                        outf[:].rearrange("p (i d) -> p i d", i=4),
                    )

            state = emit_xw(0)
            HH = H // 2
            for t in range(t_steps):
                have_x = (not is_dec) or t > 0
                have_h = hT_prev is not None
                have_xn = have_x or not zero_bias
                pr, pz, pxn, phn, emit = state

                # -- hW(t): recurrent matmuls, bank-major (r fully first:
                # sigmoid(r) heads the serial gate chain). k-chunk matmuls
                # are gated on the matching 128-col chunk of h'(t-1).
                if have_h:
                    for bank, lo in ((pr, 0), (pz, H), (phn, 2 * H)):
                        for m in range(4):
                            sl = bank[:, m * P:(m + 1) * P]
                            if bank is phn and not zero_bias:
                                emit(bank, sl, biashn[g][0:1, m * P:(m + 1) * P],
                                     ones_row[0:1, 0:P])
                            for k in range(4):
                                emit(bank, sl, whh[g][k][:, lo + m * P:lo + (m + 1) * P],
                                     hT_prev[:, k * P:(k + 1) * P])
                if t + 1 < t_steps:
                    state = emit_xw(t + 1)
                if is_dec and t > 0:
                    emit_out(t - 1, hT_prev)

                # -- gate tail (transposed layout):
                #   r = sig(pr); z = sig(pz); n = tanh(xn + r*hn)
                #   h = n + z*(h_prev - n)
                r_t = sb.tile([P, H], bf16, name="r_t", tag="r")
                z_t = sb.tile([P, H], bf16, name="z_t", tag="z")
                n_t = sb.tile([P, H], bf16, tag="n")
                e_t = sb.tile([P, H], bf16, tag="e")
                h_new = sb.tile([P, H], bf16, tag="h")
                if have_h:
                    t1 = sb.tile([P, H], bf16, tag="t1")
                    t2 = sb.tile([P, H], bf16, name="t2", tag="t2") if have_xn else t1
                    d_t = sb.tile([P, H], bf16, tag="d")
                    nc.scalar.activation(r_t[:], pr[:], AF.Sigmoid)
                nc.scalar.activation(z_t[:], pz[:], AF.Sigmoid)
                if have_h:
                    for half in range(2):
                        hs = slice(half * HH, (half + 1) * HH)
                        op = nc.vector.tensor_mul(t1[:, hs], r_t[:, hs], phn[:, hs])
                        if have_xn:
                            nc.vector.tensor_add(t2[:, hs], t1[:, hs], pxn[:, hs])
                    last_phn_reader[0] = op
                for half in range(2):
                    hs = slice(half * HH, (half + 1) * HH)
                    if have_h:
                        nc.scalar.activation(n_t[:, hs], t2[:, hs], AF.Tanh)
                        nc.vector.tensor_sub(d_t[:, hs], hT_prev[:, hs], n_t[:, hs])
                        nc.vector.tensor_mul(e_t[:, hs], z_t[:, hs], d_t[:, hs])
                        nc.vector.tensor_add(h_new[:, hs], n_t[:, hs], e_t[:, hs])
                    else:
                        nc.scalar.activation(n_t[:, hs], pxn[:, hs], AF.Tanh)
                        nc.vector.tensor_mul(e_t[:, hs], z_t[:, hs], n_t[:, hs])
                        nc.vector.tensor_sub(h_new[:, hs], n_t[:, hs], e_t[:, hs])
                hT_prev = h_new
            if is_dec:
                emit_out(t_steps - 1, hT_prev)
            return hT_prev

        with tc.tile_pool(name="gru_sb", bufs=3) as sb, \
             tc.tile_pool(name="psum_g", bufs=2, space="PSUM") as pg, \
             tc.tile_pool(name="psum_g1", bufs=1, space="PSUM") as pg1, \
             tc.tile_pool(name="psum_po", bufs=1, space="PSUM") as pout, \
             tc.tile_pool(name="ostage", bufs=2) as ostage:
            hT_enc = gru_loop("enc", False, None, sb, pg, pg1, pout, ostage)
            hid = sb.tile([P, H], bf16, tag="h")
            nc.vector.tensor_add(hid[:], hT_enc[:], noiseT[:])
            gru_loop("dec", True, hid, sb, pg, pg1, pout, ostage)

    nc.compile()
    return nc


_CACHE = {}


def _get_module(zero_bias: bool):
    key = zero_bias
    if key not in _CACHE:
        _CACHE[key] = build_module(zero_bias)
    return _CACHE[key]


def make_in_maps(past_input, future_input, noise,
                 W_emb, b_emb,
                 W_ih_enc, W_hh_enc, b_ih_enc, b_hh_enc,
                 W_ih_dec, W_hh_dec, b_ih_dec, b_hh_dec,
                 W_out, b_out):
    f = np.float32
    bf = NP_BF16
    shared = {
        "w_emb": np.asarray(W_emb, f).astype(bf),
        "b_emb": np.asarray(b_emb, f).reshape(1, E),
        "w_ih_enc": np.asarray(W_ih_enc, f).astype(bf),
        "w_hh_enc": np.asarray(W_hh_enc, f).astype(bf),
        "b_ih_enc": np.asarray(b_ih_enc, f).reshape(1, H3),
        "b_hh_enc": np.asarray(b_hh_enc, f).reshape(1, H3),
        "w_ih_dec": np.asarray(W_ih_dec, f).astype(bf),
        "w_hh_dec": np.asarray(W_hh_dec, f).astype(bf),
        "b_ih_dec": np.asarray(b_ih_dec, f).reshape(1, H3),
        "b_hh_dec": np.asarray(b_hh_dec, f).reshape(1, H3),
        "w_out": np.asarray(W_out, f).astype(bf),
        "b_out": np.asarray(b_out, f).reshape(1, D),
    }
    past_bf = np.asarray(past_input, f).astype(bf)
    fut_bf = np.asarray(future_input, f).astype(bf)
    noise_bf = np.asarray(noise, f).astype(bf)
    in_maps = []
    for c in range(NCORES):
        sl = slice(c * BL, (c + 1) * BL)
        m = dict(shared)
        m["past"] = np.ascontiguousarray(past_bf[:, sl, :])
        m["fut"] = np.ascontiguousarray(fut_bf[:, sl, :])
        m["noise"] = np.ascontiguousarray(noise_bf[sl, :])
        in_maps.append(m)
    return in_maps


def kernel(past_input, future_input, noise,
           W_emb, b_emb,
           W_ih_enc, W_hh_enc, b_ih_enc, b_hh_enc,
           W_ih_dec, W_hh_dec, b_ih_dec, b_hh_dec,
           W_out, b_out):
    zero_bias = not any(
        np.any(np.asarray(b)) for b in (b_emb, b_ih_enc, b_hh_enc, b_ih_dec, b_hh_dec, b_out)
    )
    nc = _get_module(zero_bias)
    in_maps = make_in_maps(past_input, future_input, noise,
                           W_emb, b_emb,
                           W_ih_enc, W_hh_enc, b_ih_enc, b_hh_enc,
                           W_ih_dec, W_hh_dec, b_ih_dec, b_hh_dec,
                           W_out, b_out)
    res = bass_utils.run_bass_kernel_spmd(nc, in_maps, core_ids=list(range(NCORES)))
    return np.concatenate([r["out"] for r in res.results], axis=1)


# revision 9
# speedup vs baseline: 3.1099x; 3.1099x over previous
"""Trainium2 Bass kernel for a GRU encoder-decoder (KLCPD generator).

Model (see reference):
  past_emb = relu(past @ W_emb + b_emb)            [T,B,E]
  fut_emb  = relu(future @ W_emb + b_emb)          [T,B,E]
  _, h_T   = GRU_enc(past_emb, h0=0)
  hidden   = h_T + noise
  ys, _    = GRU_dec(shift(fut_emb), h0=hidden)
  out      = ys @ W_out + b_out                    [T,B,D]

Sharding: data-parallel over batch B=1024 across 8 NeuronCores
(B_local=128); all weights replicated; no collectives.

Per-core kernel layout decisions:
  * All matmul inputs are bf16 (fp32 accumulation in PSUM); inputs and
    weights are cast to bf16 on the HOST, halving DMA traffic and
    removing all on-device staging casts.
  * The GRU hidden state is kept *transposed* in SBUF as
    hT[p, k*128 + b] = h[b, k*128 + p]  (k = H-chunk 0..3), so the
    elementwise gate math produces, with zero extra transposes, exactly
    the stationary operand needed by the next step's matmuls.
  * Per step the PE emits bank-major waves (all 16 r matmuls, then z,
    then hn, then step t+1's input projections), so sigmoid(r) — the
    head of the serial gate chain — unblocks ~0.9us into the wave.
  * Gate tail: h = n + z*(h_prev - n), all plain 2x-mode DVE
    tensor-tensor ops on bf16; sigmoids full-width, the rest H-halved.
  * The decoder's output projection out_t = h_t @ W_out rides the GRU
    loop as 4 extra matmuls per step (the transposed state IS the
    needed lhsT), accumulating 4 steps per PSUM bank; no DRAM
    round-trip of the decoder states.
"""

import os
from contextlib import ExitStack

import numpy as np

import concourse.bass as bass
import concourse.tile as tile
from concourse import bacc, bass_utils, masks, mybir
from concourse.tile_rust import add_dep_helper

T, B, D, E, H = 64, 1024, 128, 256, 512
NCORES = 8
BL = B // NCORES  # 128
H3 = 3 * H
P = 128

f32 = mybir.dt.float32
bf16 = mybir.dt.bfloat16
NP_BF16 = mybir.dt.np(bf16)
AF = mybir.ActivationFunctionType
OP = mybir.AluOpType


def _mm(nc, out, lhsT, rhs, start, stop):
    return nc.tensor.matmul(out, lhsT, rhs, start=start, stop=stop,
                            skip_group_check=True)


# The single-buffered phn bank is protected by tile's slice-level
# read/write ordering (PSUM start=True clears has_written bits, not data);
# the explicit cross-step dep deadlocks the tile scheduler.
USE_PHN_DEP = False


def build_module(zero_bias: bool, t_steps: int = T):
    """Builds the per-core Bass module. Returns the compiled nc."""
    nc = bacc.Bacc("TRN2", target_bir_lowering=False, debug=False)

    past = nc.dram_tensor("past", [t_steps, BL, D], bf16, kind="ExternalInput").ap()
    fut = nc.dram_tensor("fut", [t_steps, BL, D], bf16, kind="ExternalInput").ap()
    noise = nc.dram_tensor("noise", [BL, H], bf16, kind="ExternalInput").ap()
    w_emb = nc.dram_tensor("w_emb", [D, E], bf16, kind="ExternalInput").ap()
    b_emb = nc.dram_tensor("b_emb", [1, E], f32, kind="ExternalInput").ap()
    wd = {}
    for g in ("enc", "dec"):
        wd[g, "ih"] = nc.dram_tensor(f"w_ih_{g}", [E, H3], bf16, kind="ExternalInput").ap()
        wd[g, "hh"] = nc.dram_tensor(f"w_hh_{g}", [H, H3], bf16, kind="ExternalInput").ap()
        wd[g, "bih"] = nc.dram_tensor(f"b_ih_{g}", [1, H3], f32, kind="ExternalInput").ap()
        wd[g, "bhh"] = nc.dram_tensor(f"b_hh_{g}", [1, H3], f32, kind="ExternalInput").ap()
    w_out = nc.dram_tensor("w_out", [H, D], bf16, kind="ExternalInput").ap()
    b_out = nc.dram_tensor("b_out", [1, D], f32, kind="ExternalInput").ap()
    out = nc.dram_tensor("out", [t_steps, BL, D], f32, kind="ExternalOutput").ap()

    with tile.TileContext(nc, pool_alloc_mode="queue") as tc, ExitStack() as octx:
        wpool = octx.enter_context(tc.tile_pool(name="weights", bufs=1))

        # ---- constants -------------------------------------------------
        ident = wpool.tile([P, P], bf16)
        masks.make_identity(nc, ident[:])
        ones_row = wpool.tile([1, 512], bf16)
        nc.gpsimd.memset(ones_row[:], 1.0)

        # ---- small weights needed first (program order = DMA order) ----
        wemb_bf = wpool.tile([P, E], bf16)
        nc.sync.dma_start(wemb_bf[:], w_emb[:, :])
        nbf = wpool.tile([P, H], bf16, tag="nstage")
        nc.sync.dma_start(nbf[:], noise[:, :])

        bemb_bf = None
        if not zero_bias:
            with tc.tile_pool(name="bstage", bufs=1) as bstage:
                st = bstage.tile([1, E], f32, tag="s_bemb")
                nc.sync.dma_start(st[:], b_emb[:, :])
                bemb_bf = wpool.tile([1, E], bf16)
                nc.vector.tensor_copy(bemb_bf[:], st[:])

        # noise, transposed: noiseT[p, k*128+b] = noise[b, k*128+p]
        noiseT = wpool.tile([P, H], bf16)
        with tc.tile_pool(name="psum_noise", bufs=1, space="PSUM") as pn:
            pt = pn.tile([P, H], bf16)
            for k in range(4):
                nc.tensor.transpose(pt[:, k * P:(k + 1) * P], nbf[:, k * P:(k + 1) * P],
                                    ident[:])
            nc.scalar.copy(noiseT[:], pt[:])

        # ---- embedding precompute --------------------------------------
        # embT[g][e][p, t*BL + b] = relu(x[t] @ W_emb + b_emb)[b, e*128+p]
        # Interleaved with the bulk weight DMAs so the x transfers (needed
        # first) lead the DMA queue.
        embT = {g: [wpool.tile([P, t_steps * BL], bf16, name=f"embT_{g}_{e}",
                               tag=f"embT_{g}_{e}")
                    for e in range(2)]
                for g in ("enc", "dec")}
        whh = {}   # whh[g][k]: [128, H3]
        wih = {}   # wih[g][e]: [128, H3]
        biasx = {}  # [1, H3]  (b_ih + b_hh on r,z cols; b_ih on n cols)
        biashn = {}  # [1, 512] (b_hh n-part)
        n_grp = t_steps // 4

        def emb_phase(g, x_ap, ep, pep):
            for gi in range(n_grp):
                xs = ep.tile([P, 4 * P], bf16, tag="xs")
                nc.sync.dma_start(
                    xs[:].rearrange("p (i d) -> p i d", i=4),
                    x_ap[4 * gi:4 * gi + 4].transpose([1, 0, 2]),
                )
                ptr = pep.tile([P, 4 * P], bf16, tag="ptr")
                for i in range(4):
                    nc.tensor.transpose(ptr[:, i * P:(i + 1) * P],
                                        xs[:, i * P:(i + 1) * P], ident[:])
                xT = ep.tile([P, 4 * P], bf16, tag="xT")
                nc.scalar.copy(xT[:], ptr[:])
                for e in range(2):
                    pe_ = pep.tile([P, 4 * P], f32, tag=f"pe{e}")
                    _mm(nc, pe_[:], wemb_bf[:, e * P:(e + 1) * P], xT[:],
                        start=True, stop=zero_bias)
                    if not zero_bias:
                        _mm(nc, pe_[:], bemb_bf[0:1, e * P:(e + 1) * P],
                            ones_row[0:1, :], start=False, stop=True)
                    dst = embT[g][e][:, gi * 4 * P:(gi + 1) * 4 * P]
                    if e == 0:
                        nc.scalar.activation(dst, pe_[:], AF.Relu)
                    else:
                        nc.vector.tensor_scalar_max(dst, pe_[:], 0.0)

        def load_gru_weights(g):
            wih[g] = []
            for e in range(2):
                t_ = wpool.tile([P, H3], bf16, tag=f"wih_{g}_{e}")
                nc.sync.dma_start(t_[:], wd[g, "ih"][e * P:(e + 1) * P, :])
                wih[g].append(t_)
            whh[g] = []
            for k in range(4):
                t_ = wpool.tile([P, H3], bf16, tag=f"whh_{g}_{k}")
                nc.sync.dma_start(t_[:], wd[g, "hh"][k * P:(k + 1) * P, :])
                whh[g].append(t_)
            if not zero_bias:
                with tc.tile_pool(name=f"bstage_{g}", bufs=1) as stage:
                    sih = stage.tile([1, H3], f32, tag="s_bih")
                    shh = stage.tile([1, H3], f32, tag="s_bhh")
                    nc.sync.dma_start(sih[:], wd[g, "bih"][:, :])
                    nc.sync.dma_start(shh[:], wd[g, "bhh"][:, :])
                    bx = wpool.tile([1, H3], bf16, tag=f"biasx_{g}")
                    nc.vector.tensor_add(bx[:, 0:2 * H], sih[:, 0:2 * H], shh[:, 0:2 * H])
                    nc.vector.tensor_copy(bx[:, 2 * H:H3], sih[:, 2 * H:H3])
                    bh = wpool.tile([1, H], bf16, tag=f"biashn_{g}")
                    nc.vector.tensor_copy(bh[:], shh[:, 2 * H:H3])
                    biasx[g] = bx
                    biashn[g] = bh

        with tc.tile_pool(name="estage", bufs=4) as ep, \
             tc.tile_pool(name="psum_emb", bufs=2, space="PSUM") as pep:
            emb_phase("enc", past, ep, pep)
            load_gru_weights("enc")
            emb_phase("dec", fut, ep, pep)
            load_gru_weights("dec")
            wout_bf = wpool.tile([P, 4 * P], bf16)  # col block k = W_out rows k
            for k in range(4):
                nc.sync.dma_start(wout_bf[:, k * P:(k + 1) * P],
                                  w_out[k * P:(k + 1) * P, :])
            bout_bf = None
            if not zero_bias:
                with tc.tile_pool(name="bstage_o", bufs=1) as stage:
                    st = stage.tile([1, D], f32, tag="s_bout")
                    nc.sync.dma_start(st[:], b_out[:, :])
                    bout_bf = wpool.tile([1, D], bf16)
                    nc.vector.tensor_copy(bout_bf[:], st[:])

        # ---- GRU loops --------------------------------------------------
        last_phn_reader = [None]  # last DVE op reading the (single) phn bank

        def gru_loop(g, is_dec, hT0, sb, pg, pg1, pout, ostage):
            """Runs t_steps of GRU g. hT0 = initial transposed state (or None).
            Returns final hT tile."""
            hT_prev = hT0
            po = [None]

            def emit_xw(t):
                """Allocate step t's PSUM banks and emit its input-projection
                matmuls. Returns the bank state for the recurrent matmuls and
                gate tail."""
                have_x = (not is_dec) or t > 0
                have_h = t > 0 or hT0 is not None
                have_xn = have_x or not zero_bias
                pr = pg.tile([P, H], f32, name="pr", tag="pr")
                pz = pg.tile([P, H], f32, name="pz", tag="pz")
                pxn = pg.tile([P, H], f32, name="pxn", tag="pxn") if have_xn else None
                phn = pg1.tile([P, H], f32, name="phn", tag="phn") if have_h else None

                nbias = 0 if zero_bias else 1
                nxw = (2 if have_x else 0) + nbias
                nhw = 4 if have_h else 0
                totals = {id(pr): 4 * (nxw + nhw), id(pz): 4 * (nxw + nhw)}
                if pxn is not None:
                    totals[id(pxn)] = 4 * nxw
                if phn is not None:
                    totals[id(phn)] = 4 * (4 + nbias)
                emitted = {k: 0 for k in totals}

                def emit(bank, sl, lhsT, rhs):
                    emitted[id(bank)] += 1
                    op = _mm(nc, sl, lhsT, rhs,
                             start=emitted[id(bank)] == 1,
                             stop=emitted[id(bank)] == totals[id(bank)])
                    if (USE_PHN_DEP and bank is phn and emitted[id(bank)] == 1
                            and last_phn_reader[0] is not None):
                        # The start=True pending-zero clears the WHOLE bank;
                        # tile only tracks the written slice, so order the
                        # clear after the previous step's last phn read.
                        add_dep_helper(last_phn_reader[0].ins, op.ins, sync=True,
                                       reason="phn bank pending-zero WAR")
                    return op

                tcol = (t - 1) if is_dec else t
                lx = ([embT[g][e][:, tcol * BL:(tcol + 1) * BL] for e in range(2)]
                      if have_x else None)
                for bank, lo in ((pr, 0), (pz, H), (pxn, 2 * H)):
                    if bank is None:
                        continue
                    for m in range(4):
                        sl = bank[:, m * P:(m + 1) * P]
                        if not zero_bias:
                            emit(bank, sl, biasx[g][0:1, lo + m * P:lo + (m + 1) * P],
                                 ones_row[0:1, 0:P])
                        if have_x:
                            for e in range(2):
                                emit(bank, sl, wih[g][e][:, lo + m * P:lo + (m + 1) * P], lx[e])
                return pr, pz, pxn, phn, emit

            def emit_out(t, h_t):
                """Fused output projection: out[t] = h_t @ W_out (+ b_out),
                accumulated into quarter t%4 of the po bank."""
                i = t % 4
                if i == 0:
                    po[0] = pout.tile([P, 4 * P], f32, name="po", tag="po")
                sl = po[0][:, i * P:(i + 1) * P]
                if not zero_bias:
                    _mm(nc, sl, bout_bf[0:1, :], ones_row[0:1, 0:P],
                        start=True, stop=False)
                for k in range(4):
                    _mm(nc, sl, h_t[:, k * P:(k + 1) * P], wout_bf[:, k * P:(k + 1) * P],
                        start=zero_bias and k == 0, stop=k == 3)
                if i == 3:
                    outf = ostage.tile([P, 4 * P], f32, tag="outf")
                    nc.scalar.copy(outf[:], po[0][:])
                    w = t // 4
                    nc.sync.dma_start(
                        out[4 * w:4 * w + 4].transpose([1, 0, 2]),
                        outf[:].rearrange("p (i d) -> p i d", i=4),
                    )

            state = emit_xw(0)
            HH = H // 2
            for t in range(t_steps):
                have_x = (not is_dec) or t > 0
                have_h = hT_prev is not None
                have_xn = have_x or not zero_bias
                pr, pz, pxn, phn, emit = state

                # -- hW(t): recurrent matmuls. k∈{0,1} first (gated on the
                # first half of h'(t-1), which the tail produces early),
                # then k∈{2,3}; within each batch bank-major (r, hn, z) so
                # sigmoid(r) — the head of the serial gate chain — and then
                # r*hn unblock as early as possible. Step t+1's
                # dependency-free input projections and step t-1's output
                # projection fill the PE wait for h'(t). ------------------
                def hw_batch(kpair):
                    for bank, lo in ((pr, 0), (phn, 2 * H), (pz, H)):
                        for m in range(4):
                            sl = bank[:, m * P:(m + 1) * P]
                            if bank is phn and not zero_bias and kpair[0] == 0:
                                emit(bank, sl, biashn[g][0:1, m * P:(m + 1) * P],
                                     ones_row[0:1, 0:P])
                            for k in kpair:
                                emit(bank, sl, whh[g][k][:, lo + m * P:lo + (m + 1) * P],
                                     hT_prev[:, k * P:(k + 1) * P])

                if have_h:
                    hw_batch((0, 1))
                    hw_batch((2, 3))
                if t + 1 < t_steps:
                    state = emit_xw(t + 1)
                if is_dec and t > 0:
                    emit_out(t - 1, hT_prev)

                # -- gate tail (transposed layout):
                #   r = sig(pr); z = sig(pz); n = tanh(xn + r*hn)
                #   h = n + z*(h_prev - n)
                r_t = sb.tile([P, H], bf16, name="r_t", tag="r")
                z_t = sb.tile([P, H], bf16, name="z_t", tag="z")
                n_t = sb.tile([P, H], bf16, tag="n")
                e_t = sb.tile([P, H], bf16, tag="e")
                h_new = sb.tile([P, H], bf16, tag="h")
                H0 = slice(0, HH)
                H1 = slice(HH, H)
                if have_h:
                    t1 = sb.tile([P, H], bf16, tag="t1")
                    t2 = sb.tile([P, H], bf16, name="t2", tag="t2") if have_xn else t1
                    d_t = sb.tile([P, H], bf16, tag="d")
                    # ACT FIFO: sr0, sr1, sz0, tanh0, tanh1, sz1
                    # DVE FIFO: t1h0, t2h0, t1h1, t2h1, D0, E0, h0, D1, E1, h1
                    nc.scalar.activation(r_t[:, H0], pr[:, H0], AF.Sigmoid)
                    nc.scalar.activation(r_t[:, H1], pr[:, H1], AF.Sigmoid)
                    nc.scalar.activation(z_t[:, H0], pz[:, H0], AF.Sigmoid)
                    for hs in (H0, H1):
                        op = nc.vector.tensor_mul(t1[:, hs], r_t[:, hs], phn[:, hs])
                        if have_xn:
                            nc.vector.tensor_add(t2[:, hs], t1[:, hs], pxn[:, hs])
                    last_phn_reader[0] = op
                    nc.scalar.activation(n_t[:, H0], t2[:, H0], AF.Tanh)
                    nc.scalar.activation(n_t[:, H1], t2[:, H1], AF.Tanh)
                    nc.scalar.activation(z_t[:, H1], pz[:, H1], AF.Sigmoid)
                    for hs in (H0, H1):
                        nc.vector.tensor_sub(d_t[:, hs], hT_prev[:, hs], n_t[:, hs])
                        nc.vector.tensor_mul(e_t[:, hs], z_t[:, hs], d_t[:, hs])
                        nc.vector.tensor_add(h_new[:, hs], n_t[:, hs], e_t[:, hs])
                else:
                    nc.scalar.activation(z_t[:], pz[:], AF.Sigmoid)
                    for hs in (H0, H1):
                        nc.scalar.activation(n_t[:, hs], pxn[:, hs], AF.Tanh)
                        nc.vector.tensor_mul(e_t[:, hs], z_t[:, hs], n_t[:, hs])
                        nc.vector.tensor_sub(h_new[:, hs], n_t[:, hs], e_t[:, hs])
                hT_prev = h_new
            if is_dec:
                emit_out(t_steps - 1, hT_prev)
            return hT_prev

        with tc.tile_pool(name="gru_sb", bufs=3) as sb, \
             tc.tile_pool(name="psum_g", bufs=2, space="PSUM") as pg, \
             tc.tile_pool(name="psum_g1", bufs=1, space="PSUM") as pg1, \
             tc.tile_pool(name="psum_po", bufs=1, space="PSUM") as pout, \
             tc.tile_pool(name="ostage", bufs=2) as ostage:
            hT_enc = gru_loop("enc", False, None, sb, pg, pg1, pout, ostage)
            hid = sb.tile([P, H], bf16, tag="h")
            nc.vector.tensor_add(hid[:], hT_enc[:], noiseT[:])
            gru_loop("dec", True, hid, sb, pg, pg1, pout, ostage)

    nc.compile()
    return nc


_CACHE = {}


def _get_module(zero_bias: bool):
    key = zero_bias
    if key not in _CACHE:
        _CACHE[key] = build_module(zero_bias)
    return _CACHE[key]


def make_in_maps(past_input, future_input, noise,
                 W_emb, b_emb,
                 W_ih_enc, W_hh_enc, b_ih_enc, b_hh_enc,
                 W_ih_dec, W_hh_dec, b_ih_dec, b_hh_dec,
                 W_out, b_out):
    f = np.float32
    bf = NP_BF16
    shared = {
        "w_emb": np.asarray(W_emb, f).astype(bf),
        "b_emb": np.asarray(b_emb, f).reshape(1, E),
        "w_ih_enc": np.asarray(W_ih_enc, f).astype(bf),
        "w_hh_enc": np.asarray(W_hh_enc, f).astype(bf),
        "b_ih_enc": np.asarray(b_ih_enc, f).reshape(1, H3),
        "b_hh_enc": np.asarray(b_hh_enc, f).reshape(1, H3),
        "w_ih_dec": np.asarray(W_ih_dec, f).astype(bf),
        "w_hh_dec": np.asarray(W_hh_dec, f).astype(bf),
        "b_ih_dec": np.asarray(b_ih_dec, f).reshape(1, H3),
        "b_hh_dec": np.asarray(b_hh_dec, f).reshape(1, H3),
        "w_out": np.asarray(W_out, f).astype(bf),
        "b_out": np.asarray(b_out, f).reshape(1, D),
    }
    past_bf = np.asarray(past_input, f).astype(bf)
    fut_bf = np.asarray(future_input, f).astype(bf)
    noise_bf = np.asarray(noise, f).astype(bf)
    in_maps = []
    for c in range(NCORES):
        sl = slice(c * BL, (c + 1) * BL)
        m = dict(shared)
        m["past"] = np.ascontiguousarray(past_bf[:, sl, :])
        m["fut"] = np.ascontiguousarray(fut_bf[:, sl, :])
        m["noise"] = np.ascontiguousarray(noise_bf[sl, :])
        in_maps.append(m)
    return in_maps


def kernel(past_input, future_input, noise,
           W_emb, b_emb,
           W_ih_enc, W_hh_enc, b_ih_enc, b_hh_enc,
           W_ih_dec, W_hh_dec, b_ih_dec, b_hh_dec,
           W_out, b_out):
    zero_bias = not any(
        np.any(np.asarray(b)) for b in (b_emb, b_ih_enc, b_hh_enc, b_ih_dec, b_hh_dec, b_out)
    )
    nc = _get_module(zero_bias)
    in_maps = make_in_maps(past_input, future_input, noise,
                           W_emb, b_emb,
                           W_ih_enc, W_hh_enc, b_ih_enc, b_hh_enc,
                           W_ih_dec, W_hh_dec, b_ih_dec, b_hh_dec,
                           W_out, b_out)
    res = bass_utils.run_bass_kernel_spmd(nc, in_maps, core_ids=list(range(NCORES)))
    return np.concatenate([r["out"] for r in res.results], axis=1)


# revision 22
# speedup vs baseline: 4.3618x; 1.4026x over previous
"""Trainium2 Bass kernel for a GRU encoder-decoder (KLCPD generator).

Model (see reference):
  past_emb = relu(past @ W_emb + b_emb)            [T,B,E]
  fut_emb  = relu(future @ W_emb + b_emb)          [T,B,E]
  _, h_T   = GRU_enc(past_emb, h0=0)
  hidden   = h_T + noise
  ys, _    = GRU_dec(shift(fut_emb), h0=hidden)
  out      = ys @ W_out + b_out                    [T,B,D]

Sharding: data-parallel over batch B=1024 across 8 NeuronCores
(B_local=128); all weights replicated; no collectives.

Per-core kernel layout decisions:
  * All matmul inputs are bf16 (fp32 accumulation in PSUM); inputs and
    weights are cast to bf16 on the HOST, halving DMA traffic and
    removing all on-device staging casts.
  * The GRU hidden state is kept *transposed* in SBUF as
    hT[p, k*128 + b] = h[b, k*128 + p]  (k = H-chunk 0..3), so the
    elementwise gate math produces, with zero extra transposes, exactly
    the stationary operand needed by the next step's matmuls.
  * Per step the PE emits bank-major waves (all 16 r matmuls, then z,
    then hn, then step t+1's input projections), so sigmoid(r) — the
    head of the serial gate chain — unblocks ~0.9us into the wave.
  * Gate tail: h = n + z*(h_prev - n), all plain 2x-mode DVE
    tensor-tensor ops on bf16; sigmoids full-width, the rest H-halved.
  * The decoder's output projection out_t = h_t @ W_out rides the GRU
    loop as 4 extra matmuls per step (the transposed state IS the
    needed lhsT), accumulating 4 steps per PSUM bank; no DRAM
    round-trip of the decoder states.
"""

import os
from contextlib import ExitStack

import numpy as np

import concourse.bass as bass
import concourse.tile as tile
from concourse import bacc, bass_utils, masks, mybir
from concourse.tile_rust import add_dep_helper

T, B, D, E, H = 64, 1024, 128, 256, 512
NCORES = 8
BL = B // NCORES  # 128
H3 = 3 * H
P = 128

f32 = mybir.dt.float32
bf16 = mybir.dt.bfloat16
NP_BF16 = mybir.dt.np(bf16)
AF = mybir.ActivationFunctionType
OP = mybir.AluOpType


def _mm(nc, out, lhsT, rhs, start, stop):
    return nc.tensor.matmul(out, lhsT, rhs, start=start, stop=stop,
                            skip_group_check=True)


# The single-buffered phn bank is protected by tile's slice-level
# read/write ordering (PSUM start=True clears has_written bits, not data);
# the explicit cross-step dep deadlocks the tile scheduler.
USE_PHN_DEP = False


def build_module(zero_bias: bool, t_steps: int = T):
    """Builds the per-core Bass module. Returns the compiled nc."""
    nc = bacc.Bacc("TRN2", target_bir_lowering=False, debug=False)

    # x inputs are host-reshaped to [(t, bh), (bp, d)] so each SBUF partition
    # (t, bh) receives one fully contiguous 16KB DMA row (vs 256B gather
    # descriptors); the emb relu scatter-write restores natural batch order.
    past = nc.dram_tensor("past", [2 * t_steps, 64 * D], bf16, kind="ExternalInput").ap()
    fut = nc.dram_tensor("fut", [2 * t_steps, 64 * D], bf16, kind="ExternalInput").ap()
    noise = nc.dram_tensor("noise", [BL, H], bf16, kind="ExternalInput").ap()
    w_emb = nc.dram_tensor("w_emb", [D, E], bf16, kind="ExternalInput").ap()
    b_emb = nc.dram_tensor("b_emb", [1, E], f32, kind="ExternalInput").ap()
    wd = {}
    for g in ("enc", "dec"):
        wd[g, "ih"] = nc.dram_tensor(f"w_ih_{g}", [E, H3], bf16, kind="ExternalInput").ap()
        wd[g, "hh"] = nc.dram_tensor(f"w_hh_{g}", [H, H3], bf16, kind="ExternalInput").ap()
        wd[g, "bih"] = nc.dram_tensor(f"b_ih_{g}", [1, H3], f32, kind="ExternalInput").ap()
        wd[g, "bhh"] = nc.dram_tensor(f"b_hh_{g}", [1, H3], f32, kind="ExternalInput").ap()
    w_out = nc.dram_tensor("w_out", [H, D], bf16, kind="ExternalInput").ap()
    b_out = nc.dram_tensor("b_out", [1, D], f32, kind="ExternalInput").ap()
    out = nc.dram_tensor("out", [t_steps, BL, D], f32, kind="ExternalOutput").ap()

    with tile.TileContext(nc, pool_alloc_mode="queue") as tc, ExitStack() as octx:
        wpool = octx.enter_context(tc.tile_pool(name="weights", bufs=1))

        # ---- constants -------------------------------------------------
        ident = wpool.tile([P, P], bf16)
        masks.make_identity(nc, ident[:])
        ones_row = wpool.tile([1, 512], bf16)
        nc.gpsimd.memset(ones_row[:], 1.0)

        # ---- small weights needed first (program order = DMA order) ----
        wemb_bf = wpool.tile([P, E], bf16)
        nc.sync.dma_start(wemb_bf[:], w_emb[:, :])
        nbf = wpool.tile([P, H], bf16, tag="nstage")
        nc.sync.dma_start(nbf[:], noise[:, :])

        bemb_bf = None
        if not zero_bias:
            with tc.tile_pool(name="bstage", bufs=1) as bstage:
                st = bstage.tile([1, E], f32, tag="s_bemb")
                nc.sync.dma_start(st[:], b_emb[:, :])
                bemb_bf = wpool.tile([1, E], bf16)
                nc.vector.tensor_copy(bemb_bf[:], st[:])

        # noise, transposed: noiseT[p, k*128+b] = noise[b, k*128+p]
        noiseT = wpool.tile([P, H], bf16)
        with tc.tile_pool(name="psum_noise", bufs=1, space="PSUM") as pn:
            pt = pn.tile([P, H], bf16)
            for k in range(4):
                nc.tensor.transpose(pt[:, k * P:(k + 1) * P], nbf[:, k * P:(k + 1) * P],
                                    ident[:])
            nc.scalar.copy(noiseT[:], pt[:])

        # ---- embedding precompute --------------------------------------
        # embT[g][e][p, t*BL + b] = relu(x[t] @ W_emb + b_emb)[b, e*128+p]
        # Interleaved with the bulk weight DMAs so the x transfers (needed
        # first) lead the DMA queue.
        embT = {g: [wpool.tile([P, t_steps * BL], bf16, name=f"embT_{g}_{e}",
                               tag=f"embT_{g}_{e}")
                    for e in range(2)]
                for g in ("enc", "dec")}
        whh = {}   # whh[g][k]: [128, H3]
        wih = {}   # wih[g][e]: [128, H3]
        biasx = {}  # [1, H3]  (b_ih + b_hh on r,z cols; b_ih on n cols)
        biashn = {}  # [1, 512] (b_hh n-part)
        n_grp = t_steps // 4

        def emb_phase(g, x_ap, ep, pep):
            # One contiguous [128, 8192] load (chunked x4 to pipeline the
            # transposes), then per 4-bp group: 4 PE transposes -> xT[d, q]
            # (q = t*2+bh), 2 matmuls + relu -> embT columns bp*128 + q.
            L = ep.tile([P, t_steps * BL], bf16, name=f"L_{g}", tag="L", bufs=2)
            W4 = t_steps * BL // 4
            for c in range(4):
                nc.sync.dma_start(L[:, c * W4:(c + 1) * W4],
                                  x_ap[:, c * W4:(c + 1) * W4])
            for gi in range(n_grp):
                ptr = pep.tile([P, 4 * P], bf16, tag="ptr")
                for i in range(4):
                    bp = gi * 4 + i
                    nc.tensor.transpose(ptr[:, i * P:(i + 1) * P],
                                        L[:, bp * P:(bp + 1) * P], ident[:])
                xT = ep.tile([P, 4 * P], bf16, tag="xT")
                nc.scalar.copy(xT[:], ptr[:])
                for e in range(2):
                    pe_ = pep.tile([P, 4 * P], f32, tag=f"pe{e}")
                    _mm(nc, pe_[:], wemb_bf[:, e * P:(e + 1) * P], xT[:],
                        start=True, stop=zero_bias)
                    if not zero_bias:
                        _mm(nc, pe_[:], bemb_bf[0:1, e * P:(e + 1) * P],
                            ones_row[0:1, :], start=False, stop=True)
                    # Scatter the relu write so embT columns come out
                    # t-contiguous (col = (t*2+bh)*64 + bp = t*128 + b):
                    # the strided write costs a little preamble ACT/DVE
                    # time, keeping the hot-loop xw matmul reads dense.
                    dst = (embT[g][e][:]
                           .rearrange("p (q bp) -> p q bp", bp=64)
                           [:, :, gi * 4:(gi + 1) * 4])
                    src = pe_[:].rearrange("p (i q) -> p q i", i=4)
                    if e == 0:
                        nc.scalar.activation(dst, src, AF.Relu)
                    else:
                        nc.vector.tensor_scalar_max(dst, src, 0.0)

        def load_gru_weights(g):
            wih[g] = []
            for e in range(2):
                t_ = wpool.tile([P, H3], bf16, tag=f"wih_{g}_{e}")
                nc.sync.dma_start(t_[:], wd[g, "ih"][e * P:(e + 1) * P, :])
                wih[g].append(t_)
            whh[g] = []
            for k in range(4):
                t_ = wpool.tile([P, H3], bf16, tag=f"whh_{g}_{k}")
                nc.sync.dma_start(t_[:], wd[g, "hh"][k * P:(k + 1) * P, :])
                whh[g].append(t_)
            if not zero_bias:
                with tc.tile_pool(name=f"bstage_{g}", bufs=1) as stage:
                    sih = stage.tile([1, H3], f32, tag="s_bih")
                    shh = stage.tile([1, H3], f32, tag="s_bhh")
                    nc.sync.dma_start(sih[:], wd[g, "bih"][:, :])
                    nc.sync.dma_start(shh[:], wd[g, "bhh"][:, :])
                    bx = wpool.tile([1, H3], bf16, tag=f"biasx_{g}")
                    nc.vector.tensor_add(bx[:, 0:2 * H], sih[:, 0:2 * H], shh[:, 0:2 * H])
                    nc.vector.tensor_copy(bx[:, 2 * H:H3], sih[:, 2 * H:H3])
                    bh = wpool.tile([1, H], bf16, tag=f"biashn_{g}")
                    nc.vector.tensor_copy(bh[:], shh[:, 2 * H:H3])
                    biasx[g] = bx
                    biashn[g] = bh

        with tc.tile_pool(name="estage", bufs=4) as ep, \
             tc.tile_pool(name="psum_emb", bufs=2, space="PSUM") as pep:
            emb_phase("enc", past, ep, pep)
            load_gru_weights("enc")
            emb_phase("dec", fut, ep, pep)
            load_gru_weights("dec")
            wout_bf = wpool.tile([P, 4 * P], bf16)  # col block k = W_out rows k
            for k in range(4):
                nc.sync.dma_start(wout_bf[:, k * P:(k + 1) * P],
                                  w_out[k * P:(k + 1) * P, :])
            bout_bf = None
            if not zero_bias:
                with tc.tile_pool(name="bstage_o", bufs=1) as stage:
                    st = stage.tile([1, D], f32, tag="s_bout")
                    nc.sync.dma_start(st[:], b_out[:, :])
                    bout_bf = wpool.tile([1, D], bf16)
                    nc.vector.tensor_copy(bout_bf[:], st[:])

        # ---- GRU loops --------------------------------------------------
        last_phn_reader = [None]  # last DVE op reading the (single) phn bank

        def gru_loop(g, is_dec, hT0, sb, pg, pg1, pout, ostage):
            """Runs t_steps of GRU g. hT0 = initial transposed state (or None).
            Returns final hT tile."""
            hT_prev = hT0
            po = [None]

            def emit_xw(t):
                """Allocate step t's PSUM banks and emit its input-projection
                matmuls. Returns the bank state for the recurrent matmuls and
                gate tail."""
                have_x = (not is_dec) or t > 0
                have_h = t > 0 or hT0 is not None
                have_xn = have_x or not zero_bias
                pr = pg.tile([P, H], f32, name="pr", tag="pr")
                pz = pg.tile([P, H], f32, name="pz", tag="pz")
                pxn = pg.tile([P, H], f32, name="pxn", tag="pxn") if have_xn else None
                phn = pg1.tile([P, H], f32, name="phn", tag="phn") if have_h else None

                nbias = 0 if zero_bias else 1
                nxw = (2 if have_x else 0) + nbias
                nhw = 4 if have_h else 0
                totals = {id(pr): 4 * (nxw + nhw), id(pz): 4 * (nxw + nhw)}
                if pxn is not None:
                    totals[id(pxn)] = 4 * nxw
                if phn is not None:
                    totals[id(phn)] = 4 * (4 + nbias)
                emitted = {k: 0 for k in totals}

                def emit(bank, sl, lhsT, rhs):
                    emitted[id(bank)] += 1
                    op = _mm(nc, sl, lhsT, rhs,
                             start=emitted[id(bank)] == 1,
                             stop=emitted[id(bank)] == totals[id(bank)])
                    if (USE_PHN_DEP and bank is phn and emitted[id(bank)] == 1
                            and last_phn_reader[0] is not None):
                        # The start=True pending-zero clears the WHOLE bank;
                        # tile only tracks the written slice, so order the
                        # clear after the previous step's last phn read.
                        add_dep_helper(last_phn_reader[0].ins, op.ins, sync=True,
                                       reason="phn bank pending-zero WAR")
                    return op

                tcol = (t - 1) if is_dec else t
                lx = ([embT[g][e][:, tcol * BL:(tcol + 1) * BL] for e in range(2)]
                      if have_x else None)
                # xw bank order matches the order the previous step's tail
                # releases each bank's buffer: sig(r) first, t2 (xn) mid,
                # sig(z1) last.
                for bank, lo in ((pr, 0), (pxn, 2 * H), (pz, H)):
                    if bank is None:
                        continue
                    for m in range(4):
                        sl = bank[:, m * P:(m + 1) * P]
                        if not zero_bias:
                            emit(bank, sl, biasx[g][0:1, lo + m * P:lo + (m + 1) * P],
                                 ones_row[0:1, 0:P])
                        if have_x:
                            for e in range(2):
                                emit(bank, sl, wih[g][e][:, lo + m * P:lo + (m + 1) * P], lx[e])
                return pr, pz, pxn, phn, emit

            def emit_out(t, h_t):
                """Fused output projection: out[t] = h_t @ W_out (+ b_out),
                accumulated into quarter t%4 of the po bank."""
                i = t % 4
                if i == 0:
                    po[0] = pout.tile([P, 4 * P], f32, name="po", tag="po")
                sl = po[0][:, i * P:(i + 1) * P]
                if not zero_bias:
                    _mm(nc, sl, bout_bf[0:1, :], ones_row[0:1, 0:P],
                        start=True, stop=False)
                for k in range(4):
                    _mm(nc, sl, h_t[:, k * P:(k + 1) * P], wout_bf[:, k * P:(k + 1) * P],
                        start=zero_bias and k == 0, stop=k == 3)
                if i == 3:
                    outf = ostage.tile([P, 4 * P], f32, tag="outf")
                    nc.scalar.copy(outf[:], po[0][:])
                    w = t // 4
                    nc.sync.dma_start(
                        out[4 * w:4 * w + 4].transpose([1, 0, 2]),
                        outf[:].rearrange("p (i d) -> p i d", i=4),
                    )

            state = emit_xw(0)
            HH = H // 2
            for t in range(t_steps):
                have_x = (not is_dec) or t > 0
                have_h = hT_prev is not None
                have_xn = have_x or not zero_bias
                pr, pz, pxn, phn, emit = state

                # -- hW(t): recurrent matmuls. k∈{0,1} first (gated on the
                # first half of h'(t-1), which the tail produces early),
                # then k∈{2,3}; within each batch bank-major (r, hn, z) so
                # sigmoid(r) — the head of the serial gate chain — and then
                # r*hn unblock as early as possible. Step t+1's
                # dependency-free input projections and step t-1's output
                # projection fill the PE wait for h'(t). ------------------
                if have_h:
                    # Batch order: pr-k01, phn-k01, pr-k23, phn-k23, pz-k01,
                    # pz-k23. pr (the gate-chain head) completes at ~1.35us;
                    # the phn-k01 batch between the two pr halves absorbs the
                    # wait for h'(t-1)'s second half (k∈{2,3} matmuls are
                    # gated on it, and it lands ~0.8us into the wave).
                    def hw_batch(bank, lo, kpair, bias):
                        for m in range(4):
                            sl = bank[:, m * P:(m + 1) * P]
                            if bias:
                                emit(bank, sl, biashn[g][0:1, m * P:(m + 1) * P],
                                     ones_row[0:1, 0:P])
                            for k in kpair:
                                emit(bank, sl,
                                     whh[g][k][:, lo + m * P:lo + (m + 1) * P],
                                     hT_prev[:, k * P:(k + 1) * P])

                    hw_batch(pr, 0, (0, 1), False)
                    hw_batch(phn, 2 * H, (0, 1), not zero_bias)
                    hw_batch(pr, 0, (2, 3), False)
                    hw_batch(phn, 2 * H, (2, 3), False)
                    hw_batch(pz, H, (0, 1), False)
                    hw_batch(pz, H, (2, 3), False)
                if t + 1 < t_steps:
                    state = emit_xw(t + 1)
                if is_dec and t > 0:
                    emit_out(t - 1, hT_prev)

                # -- gate tail (transposed layout):
                #   r = sig(pr); z = sig(pz); n = tanh(xn + r*hn)
                #   h = n + z*(h_prev - n)
                r_t = sb.tile([P, H], bf16, name="r_t", tag="r")
                z_t = sb.tile([P, H], bf16, name="z_t", tag="z")
                n_t = sb.tile([P, H], bf16, tag="n")
                e_t = sb.tile([P, H], bf16, tag="e")
                h_new = sb.tile([P, H], bf16, tag="h")
                H0 = slice(0, HH)
                H1 = slice(HH, H)
                if have_h:
                    t1 = sb.tile([P, H], bf16, tag="t1")
                    t2 = sb.tile([P, H], bf16, name="t2", tag="t2") if have_xn else t1
                    d_t = sb.tile([P, H], bf16, tag="d")
                    # ACT FIFO: sr0, sr1, tanh0, sz0, tanh1, sz1
                    # DVE FIFO: t1h0, t2h0, t1h1, t2h1, D0, E0, h0, D1, E1, h1
                    nc.scalar.activation(r_t[:, H0], pr[:, H0], AF.Sigmoid)
                    nc.scalar.activation(r_t[:, H1], pr[:, H1], AF.Sigmoid)
                    for hs in (H0, H1):
                        op = nc.vector.tensor_mul(t1[:, hs], r_t[:, hs], phn[:, hs])
                        if have_xn:
                            nc.vector.tensor_add(t2[:, hs], t1[:, hs], pxn[:, hs])
                    last_phn_reader[0] = op
                    nc.scalar.activation(n_t[:, H0], t2[:, H0], AF.Tanh)
                    nc.scalar.activation(z_t[:, H0], pz[:, H0], AF.Sigmoid)
                    nc.scalar.activation(n_t[:, H1], t2[:, H1], AF.Tanh)
                    nc.scalar.activation(z_t[:, H1], pz[:, H1], AF.Sigmoid)
                    for hs in (H0, H1):
                        nc.vector.tensor_sub(d_t[:, hs], hT_prev[:, hs], n_t[:, hs])
                        nc.vector.tensor_mul(e_t[:, hs], z_t[:, hs], d_t[:, hs])
                        nc.vector.tensor_add(h_new[:, hs], n_t[:, hs], e_t[:, hs])
                else:
                    nc.scalar.activation(z_t[:], pz[:], AF.Sigmoid)
                    for hs in (H0, H1):
                        nc.scalar.activation(n_t[:, hs], pxn[:, hs], AF.Tanh)
                        nc.vector.tensor_mul(e_t[:, hs], z_t[:, hs], n_t[:, hs])
                        nc.vector.tensor_sub(h_new[:, hs], n_t[:, hs], e_t[:, hs])
                hT_prev = h_new
            if is_dec:
                emit_out(t_steps - 1, hT_prev)
            return hT_prev

        with tc.tile_pool(name="gru_sb", bufs=3) as sb, \
             tc.tile_pool(name="psum_g", bufs=2, space="PSUM") as pg, \
             tc.tile_pool(name="psum_g1", bufs=1, space="PSUM") as pg1, \
             tc.tile_pool(name="psum_po", bufs=1, space="PSUM") as pout, \
             tc.tile_pool(name="ostage", bufs=2) as ostage:
            hT_enc = gru_loop("enc", False, None, sb, pg, pg1, pout, ostage)
            hid = sb.tile([P, H], bf16, tag="h")
            nc.vector.tensor_add(hid[:], hT_enc[:], noiseT[:])
            gru_loop("dec", True, hid, sb, pg, pg1, pout, ostage)

    nc.compile()
    return nc


_CACHE = {}


def _get_module(zero_bias: bool):
    key = zero_bias
    if key not in _CACHE:
        _CACHE[key] = build_module(zero_bias)
    return _CACHE[key]


def make_in_maps(past_input, future_input, noise,
                 W_emb, b_emb,
                 W_ih_enc, W_hh_enc, b_ih_enc, b_hh_enc,
                 W_ih_dec, W_hh_dec, b_ih_dec, b_hh_dec,
                 W_out, b_out):
    f = np.float32
    bf = NP_BF16
    shared = {
        "w_emb": np.asarray(W_emb, f).astype(bf),
        "b_emb": np.asarray(b_emb, f).reshape(1, E),
        "w_ih_enc": np.asarray(W_ih_enc, f).astype(bf),
        "w_hh_enc": np.asarray(W_hh_enc, f).astype(bf),
        "b_ih_enc": np.asarray(b_ih_enc, f).reshape(1, H3),
        "b_hh_enc": np.asarray(b_hh_enc, f).reshape(1, H3),
        "w_ih_dec": np.asarray(W_ih_dec, f).astype(bf),
        "w_hh_dec": np.asarray(W_hh_dec, f).astype(bf),
        "b_ih_dec": np.asarray(b_ih_dec, f).reshape(1, H3),
        "b_hh_dec": np.asarray(b_hh_dec, f).reshape(1, H3),
        "w_out": np.asarray(W_out, f).astype(bf),
        "b_out": np.asarray(b_out, f).reshape(1, D),
    }
    past_bf = np.asarray(past_input, f).astype(bf)
    fut_bf = np.asarray(future_input, f).astype(bf)
    noise_bf = np.asarray(noise, f).astype(bf)
    in_maps = []
    for c in range(NCORES):
        sl = slice(c * BL, (c + 1) * BL)
        m = dict(shared)
        m["past"] = np.ascontiguousarray(past_bf[:, sl, :]).reshape(2 * T, 64 * D)
        m["fut"] = np.ascontiguousarray(fut_bf[:, sl, :]).reshape(2 * T, 64 * D)
        m["noise"] = np.ascontiguousarray(noise_bf[sl, :])
        in_maps.append(m)
    return in_maps


def fix_out(dev_out):
    """Batch axis comes out in natural order (the emb relu scatter-write
    restores it); kept for harness compatibility."""
    return dev_out


def kernel(past_input, future_input, noise,
           W_emb, b_emb,
           W_ih_enc, W_hh_enc, b_ih_enc, b_hh_enc,
           W_ih_dec, W_hh_dec, b_ih_dec, b_hh_dec,
           W_out, b_out):
    zero_bias = not any(
        np.any(np.asarray(b)) for b in (b_emb, b_ih_enc, b_hh_enc, b_ih_dec, b_hh_dec, b_out)
    )
    nc = _get_module(zero_bias)
    in_maps = make_in_maps(past_input, future_input, noise,
                           W_emb, b_emb,
                           W_ih_enc, W_hh_enc, b_ih_enc, b_hh_enc,
                           W_ih_dec, W_hh_dec, b_ih_dec, b_hh_dec,
                           W_out, b_out)
    res = bass_utils.run_bass_kernel_spmd(nc, in_maps, core_ids=list(range(NCORES)))
    return np.concatenate([fix_out(r["out"]) for r in res.results], axis=1)
